# revision 9
# baseline (speedup 1.0000x reference)
"""MoE top-2 gating kernel for Trainium2 (8 NeuronCores, data-parallel).

logits = x @ W.T + b          [N=131072, E=64]
top2 -> softmax(top2 vals) scattered back into a sparse [N, E] output.

Sharding: x split along tokens into 8 shards of 16384; W/b replicated.
Each shard is pre-transposed on the host so DMA loads put the contraction
dim d on partitions (no on-chip transpose needed).
"""

import sys
from concurrent.futures import ThreadPoolExecutor

import numpy as np

for _p in ("/opt/trn_rl_repo", "/root/.axon_site/_ro/trn_rl_repo"):
    if _p not in sys.path:
        sys.path.insert(0, _p)

import concourse.bacc as bacc
import concourse.bass as bass
import concourse.mybir as mybir
from concourse.bass_utils import run_bass_kernel_spmd
from concourse.tile import TileContext

N_TOKENS = 131072
D_MODEL = 1024
NUM_EXPERTS = 64
N_CORES = 8
S = N_TOKENS // N_CORES          # tokens per core = 16384
GROUP = 512                      # tokens per DMA group
N_GROUPS = S // GROUP            # 32
SUB = GROUP // 128               # 4 sub-tiles of 128 tokens
DK = D_MODEL // 128              # 8 contraction chunks

F32 = mybir.dt.float32
U32 = mybir.dt.uint32
I32 = mybir.dt.int32

_CACHE: dict = {}


def _build_bass() -> bass.Bass:
    nc = bacc.Bacc(None, target_bir_lowering=False, debug=False)
    xT = nc.declare_dram_parameter("xT", [D_MODEL, S], F32, isOutput=False)
    wT = nc.declare_dram_parameter("wT", [D_MODEL, NUM_EXPERTS], F32, isOutput=False)
    bb = nc.declare_dram_parameter("b", [1, NUM_EXPERTS], F32, isOutput=False)
    out = nc.declare_dram_parameter("out", [S, NUM_EXPERTS], F32, isOutput=True)

    E = NUM_EXPERTS
    with TileContext(nc) as tc:
        with (
            tc.tile_pool(name="const", bufs=1) as cpool,
            tc.tile_pool(name="xin", bufs=3) as xin,
            tc.tile_pool(name="sb", bufs=4) as sb,
            tc.tile_pool(name="ps", bufs=4, space="PSUM") as pp,
        ):
            # --- constants ---
            wt_sb = cpool.tile([128, DK * E], F32)        # 8 chunks of W.T side by side
            nc.sync.dma_start(
                out=wt_sb[:, :].rearrange("p (k e) -> p k e", k=DK),
                in_=wT[:, :].rearrange("(k p) e -> p k e", p=128),
            )
            b_sb = cpool.tile([1, E], F32)
            nc.sync.dma_start(out=b_sb, in_=bb[:, :])
            ones = cpool.tile([1, 128], F32)
            nc.vector.memset(ones, 1.0)
            iota_i = cpool.tile([128, E], I32)
            nc.gpsimd.iota(iota_i, pattern=[[1, E]], channel_multiplier=0)
            iota_f = cpool.tile([128, E], F32)
            nc.vector.tensor_copy(iota_f, iota_i)
            # bias broadcast to all 128 partitions via K=1 matmul
            bias_ps = pp.tile([128, E], F32)
            nc.tensor.matmul(bias_ps, lhsT=ones, rhs=b_sb, start=True, stop=True)
            bias_sb = cpool.tile([128, E], F32)
            nc.vector.tensor_copy(bias_sb, bias_ps)

            for g in range(N_GROUPS):
                xt = xin.tile([128, DK * GROUP], F32)
                half = DK // 2 * GROUP
                for h in range(2):
                    nc.sync.dma_start(
                        out=xt[:, h * half:(h + 1) * half].rearrange(
                            "p (k t) -> p k t", k=DK // 2
                        ),
                        in_=xT[
                            h * 512:(h + 1) * 512, g * GROUP:(g + 1) * GROUP
                        ].rearrange("(k p) t -> p k t", p=128),
                    )
                for s in range(SUB):
                    ps = pp.tile([128, E], F32)
                    for k in range(DK):
                        c0 = k * GROUP + s * 128
                        nc.tensor.matmul(
                            ps,
                            lhsT=xt[:, c0:c0 + 128],
                            rhs=wt_sb[:, k * E:(k + 1) * E],
                            start=(k == 0),
                            stop=(k == DK - 1),
                        )
                    lg = sb.tile([128, E], F32)
                    nc.vector.tensor_tensor(lg, ps, bias_sb, mybir.AluOpType.add)
                    mx = sb.tile([128, 8], F32)
                    ix = sb.tile([128, 8], U32)
                    nc.vector.max(mx, lg)
                    nc.vector.max_index(ix, mx, lg)
                    ixf = sb.tile([128, 2], F32)
                    nc.gpsimd.tensor_copy(ixf, ix[:, 0:2])
                    d2 = sb.tile([128, 1], F32)
                    nc.vector.tensor_tensor(
                        d2, mx[:, 1:2], mx[:, 0:1], mybir.AluOpType.subtract
                    )
                    # softmax over the two top values: g2 = sigmoid(m2-m1), g1 = sigmoid(m1-m2)
                    g2 = sb.tile([128, 1], F32)
                    nc.scalar.activation(g2, d2, mybir.ActivationFunctionType.Sigmoid)
                    g1 = sb.tile([128, 1], F32)
                    nc.scalar.activation(
                        g1, d2, mybir.ActivationFunctionType.Sigmoid, scale=-1.0
                    )
                    o1 = sb.tile([128, E], F32)
                    nc.vector.tensor_scalar(
                        o1, iota_f, ixf[:, 0:1], g1,
                        mybir.AluOpType.is_equal, mybir.AluOpType.mult,
                    )
                    o2 = sb.tile([128, E], F32)
                    nc.vector.tensor_scalar(
                        o2, iota_f, ixf[:, 1:2], g2,
                        mybir.AluOpType.is_equal, mybir.AluOpType.mult,
                    )
                    oo = sb.tile([128, E], F32)
                    nc.gpsimd.tensor_tensor(oo, o1, o2, mybir.AluOpType.add)
                    r0 = (g * SUB + s) * 128
                    nc.sync.dma_start(out=out[r0:r0 + 128, :], in_=oo)
    nc.compile()
    return nc


def _prep_inputs(x: np.ndarray, W: np.ndarray, b: np.ndarray):
    wT = np.ascontiguousarray(W.T.astype(np.float32, copy=False))
    bb = np.ascontiguousarray(b.astype(np.float32, copy=False)).reshape(1, NUM_EXPERTS)

    def shard(c):
        return np.ascontiguousarray(x[c * S:(c + 1) * S, :].T)

    with ThreadPoolExecutor(N_CORES) as tp:
        shards = list(tp.map(shard, range(N_CORES)))
    return [{"xT": shards[c], "wT": wT, "b": bb} for c in range(N_CORES)]


def _run(x, W, b, trace=False):
    if "nc" not in _CACHE:
        _CACHE["nc"] = _build_bass()
    nc = _CACHE["nc"]
    in_maps = _prep_inputs(
        np.asarray(x, dtype=np.float32),
        np.asarray(W, dtype=np.float32),
        np.asarray(b, dtype=np.float32),
    )
    res = run_bass_kernel_spmd(nc, in_maps, list(range(N_CORES)), trace=trace)
    outs = [np.asarray(res.results[c]["out"]) for c in range(N_CORES)]
    return np.concatenate(outs, axis=0), res


def kernel(x, W, b):
    out, _ = _run(x, W, b, trace=False)
    return out
